# revision 2
# baseline (speedup 1.0000x reference)
"""Int8RouterLinear TRN2 kernel: out[16384, 64] = x[16384, 4096] @ (W_int8 * scale)^T.

Strategy (data-parallel over 8 NeuronCores, 2048 tokens each):
  - The host pre-transposes and fp16-casts each core's x shard into
    [hidden, tokens] tile layout (h on partitions), so the device does a
    pure streaming matmul — no on-chip transposes, no PSUM round-trips.
  - x^T groups [128h, 2x2048t] fp16 DMA HBM->SBUF (8KB partition lines,
    1MB per transfer, alternating the two HWDGE rings) — 16MB/core, which
    is the memory roofline for this kernel.
  - The dequantized router weight is stationary: fp16 keeps the int8
    weight values exact (<= 2048 representable), only scale multiplication
    and x round to 11 bits -> ~3e-4 rel err (measured), gate is 2e-2.
  - matmul accumulates out^T[64e, 512t] chunks in fp32 PSUM over the 32
    h-tiles (4 PSUM banks, one per 512-token chunk of the shard).
  - DVE copies PSUM->SBUF once at the end; a single 512KB DMA stores
    out^T[64, 2048] f32; the host transposes/concats (4MB total).
"""
import numpy as np

import concourse.mybir as mybir
from concourse import bacc
from concourse.tile import TileContext
from concourse.bass_utils import run_bass_kernel_spmd

TOKENS = 16384
HIDDEN = 4096
EXPERTS = 64
NCORES = 8
TSHARD = TOKENS // NCORES          # 2048 tokens per core
HT = HIDDEN // 128                 # 32 h-tiles of 128
KGRP = 2                           # h-tiles per DMA (1MB transfers)
CHUNK = 512                        # tokens per PSUM accumulation bank
NCHUNK = TSHARD // CHUNK           # 4

F32 = mybir.dt.float32
F16 = mybir.dt.float16

_cache = {}


def _build():
    if "nc" in _cache:
        return _cache["nc"]

    nc = bacc.Bacc("TRN2", target_bir_lowering=False, debug=False,
                   num_devices=NCORES)
    x_d = nc.dram_tensor("x", [HT // KGRP, 128, KGRP * TSHARD], F16,
                         kind="ExternalInput")
    w_d = nc.dram_tensor("w", [128, HT * EXPERTS], F16, kind="ExternalInput")
    o_d = nc.dram_tensor("out", [EXPERTS, TSHARD], F32, kind="ExternalOutput")

    with TileContext(nc) as tc:
        with tc.tile_pool(name="consts", bufs=1) as cpool, \
             tc.tile_pool(name="xp", bufs=4) as xpool, \
             tc.tile_pool(name="pso", bufs=1, space="PSUM") as ppool, \
             tc.tile_pool(name="ost", bufs=1) as opool:
            w_sb = cpool.tile([128, HT * EXPERTS], F16)
            nc.sync.dma_start(out=w_sb, in_=w_d.ap())
            w_v = w_sb.rearrange("p (k e) -> p k e", e=EXPERTS)

            po = [ppool.tile([EXPERTS, CHUNK], F32, name=f"po{c}",
                             tag=f"po{c}") for c in range(NCHUNK)]

            for g in range(HT // KGRP):
                xg = xpool.tile([128, KGRP * TSHARD], F16)
                eng = nc.sync if g % 2 == 0 else nc.scalar
                eng.dma_start(out=xg, in_=x_d.ap()[g])
                xv = xg.rearrange("p (j t) -> p j t", j=KGRP)
                for j in range(KGRP):
                    k = g * KGRP + j
                    for c in range(NCHUNK):
                        nc.tensor.matmul(
                            po[c], w_v[:, k, :],
                            xv[:, j, c * CHUNK:(c + 1) * CHUNK],
                            start=(k == 0), stop=(k == HT - 1))

            ot = opool.tile([EXPERTS, TSHARD], F32)
            for c in range(NCHUNK):
                nc.vector.tensor_copy(ot[:, c * CHUNK:(c + 1) * CHUNK], po[c])
            nc.sync.dma_start(out=o_d.ap(), in_=ot)

    nc.compile()
    _cache["nc"] = nc
    return nc


def _prep_w(weights_int8, scales):
    """[64, 4096] int8-valued weights -> [128, HT*EXPERTS] fp16 with
    w_arr[p, k*64 + e] = W[e, 128k + p]."""
    w = weights_int8.astype(np.float32) * scales.astype(np.float32)[:, None]
    wt = w.T.astype(np.float16)                      # [HIDDEN, EXPERTS]
    arr = wt.reshape(HT, 128, EXPERTS).transpose(1, 0, 2)
    return np.ascontiguousarray(arr).reshape(128, HT * EXPERTS)


def _prep_x(x):
    """Transpose + fp16-cast x into per-core [HT//KGRP, 128, KGRP*TSHARD]
    arrays with x_c[g, p, j*TSHARD + t] = x[c*TSHARD + t, 128*(KGRP*g+j) + p]."""
    x16 = x.astype(np.float16)
    xt = np.empty((HIDDEN, TOKENS), dtype=np.float16)
    blk = 512
    for i in range(0, TOKENS, blk):                  # blocked: cache-friendly
        xt[:, i:i + blk] = x16[i:i + blk].T
    shards = []
    for c in range(NCORES):
        sh = xt[:, c * TSHARD:(c + 1) * TSHARD]
        sh = sh.reshape(HT // KGRP, KGRP, 128, TSHARD).transpose(0, 2, 1, 3)
        shards.append(np.ascontiguousarray(sh).reshape(
            HT // KGRP, 128, KGRP * TSHARD))
    return shards


def kernel(x, weights_int8, scales):
    nc = _build()
    warr = _prep_w(np.asarray(weights_int8), np.asarray(scales))
    xshards = _prep_x(np.ascontiguousarray(x, dtype=np.float32))
    in_maps = [{"x": xshards[c], "w": warr} for c in range(NCORES)]
    res = run_bass_kernel_spmd(nc, in_maps, core_ids=list(range(NCORES)))
    out = np.concatenate(
        [res.results[c]["out"].T for c in range(NCORES)], axis=0)
    return np.ascontiguousarray(out, dtype=np.float32)
